# revision 1
# baseline (speedup 1.0000x reference)
"""Poincare pairwise edge generator on 8 Trainium2 NeuronCores.

Math: for the Poincare-ball distance with c=1, the mobius-norm numerator
factors exactly:  num2 = A^2|x|^2 - 2AB<x,y> + B^2|y|^2 = sqdist * D with
sqdist = |x-y|^2 and D = 1 - 2<x,y> + |x|^2|y|^2, so
  z = sqrt(sqdist/D) = exp(0.5*(ln sqdist - ln D))
  dists = ln(1+z) - ln(1-z)           (= 2 artanh z)
  probs = sigmoid(-dists) = (1-z)/2   (exact simplification)

-2<x_i,x_j> comes from a K=256 float32r matmul; u=|x_j|^2 is added via a
broadcast tile on DVE, s=|x_i|^2 via the activation bias port. The whole
per-element chain is 5 ACT ops + 4 DVE ops, pipelined in raw bass with
one-phase skew so cross-engine semaphore latency overlaps.

Sharding: rows across 8 cores (1024 each); every core holds the full
[256, 8192] transposed embeddings. Diagonal fixed on host at unshard
(probs diagonal is exactly 0; reference dists diagonal is fp32
cancellation noise of order 1e-4 around a true value of 0).
"""

import sys

sys.path.insert(0, '/opt/trn_rl_repo')

import numpy as np

_compiled = None


N_TOTAL = 8192
ROWS_PER_CORE = 1024
M_BLOCKS = 8
HALF = 4096
PHASES = 16


def _build_raw(reps=1, bench=False, tiny_io=False):
    import concourse.bass as bass
    import concourse.mybir as mybir

    DT = mybir.dt.float32
    DTR = mybir.dt.float32r
    F = mybir.ActivationFunctionType
    OP = mybir.AluOpType

    nc = bass.Bass()

    if tiny_io:
        nc.declare_dram_parameter("tiny", [128, 4], DT, isOutput=False)
        eta = nc.dram_tensor("eta", [128, N_TOTAL], DTR)
        etb = nc.dram_tensor("etb", [128, N_TOTAL], DTR)
        la = nc.dram_tensor("la", [128, ROWS_PER_CORE], DTR)
        lb = nc.dram_tensor("lb", [128, ROWS_PER_CORE], DTR)
        ubc = nc.dram_tensor("ubc", [128, N_TOTAL], DT)
        svec = nc.dram_tensor("svec", [128, 8], DT)
        dists_o = nc.dram_tensor("dists_i", [ROWS_PER_CORE, N_TOTAL], DT)
        probs_o = nc.dram_tensor("probs_i", [ROWS_PER_CORE, N_TOTAL], DT)
        done_o = nc.declare_dram_parameter("done_o", [128, 4], DT, isOutput=True)
    else:
        eta = nc.declare_dram_parameter("eta", [128, N_TOTAL], DTR, isOutput=False)
        etb = nc.declare_dram_parameter("etb", [128, N_TOTAL], DTR, isOutput=False)
        la = nc.declare_dram_parameter("la", [128, ROWS_PER_CORE], DTR, isOutput=False)
        lb = nc.declare_dram_parameter("lb", [128, ROWS_PER_CORE], DTR, isOutput=False)
        ubc = nc.declare_dram_parameter("ubc", [128, N_TOTAL], DT, isOutput=False)
        svec = nc.declare_dram_parameter("svec", [128, 8], DT, isOutput=False)
        dists_o = nc.declare_dram_parameter(
            "dists_o", [ROWS_PER_CORE, N_TOTAL], DT, isOutput=True)
        probs_o = nc.declare_dram_parameter(
            "probs_o", [ROWS_PER_CORE, N_TOTAL], DT, isOutput=True)
        done_o = None

    NIN = 6 * 16
    TOT = PHASES * reps

    def phase_mcol(p):
        q = p % PHASES
        return q // 2, (q % 2) * HALF

    from contextlib import ExitStack
    with ExitStack() as ctx:
        block = ctx.enter_context(nc.Block())
        dma_in = ctx.enter_context(nc.semaphore("dma_in"))
        pe_s = ctx.enter_context(nc.semaphore("pe_s"))
        dr_s = ctx.enter_context(nc.semaphore("dr_s"))
        ln_s = ctx.enter_context(nc.semaphore("ln_s"))
        h_s = ctx.enter_context(nc.semaphore("h_s"))
        z_s = ctx.enter_context(nc.semaphore("z_s"))
        o_s = ctx.enter_context(nc.semaphore("o_s"))
        dma_o = ctx.enter_context(nc.semaphore("dma_o"))
        t_eta = ctx.enter_context(nc.sbuf_tensor("t_eta", [128, N_TOTAL], DTR))
        t_etb = ctx.enter_context(nc.sbuf_tensor("t_etb", [128, N_TOTAL], DTR))
        t_la = ctx.enter_context(nc.sbuf_tensor("t_la", [128, ROWS_PER_CORE], DTR))
        t_lb = ctx.enter_context(nc.sbuf_tensor("t_lb", [128, ROWS_PER_CORE], DTR))
        t_ubc = ctx.enter_context(nc.sbuf_tensor("t_ubc", [128, N_TOTAL], DT))
        t_svec = ctx.enter_context(nc.sbuf_tensor("t_svec", [128, 8], DT))
        A0 = ctx.enter_context(nc.sbuf_tensor("A0", [128, HALF], DT))
        A1 = ctx.enter_context(nc.sbuf_tensor("A1", [128, HALF], DT))
        B0 = ctx.enter_context(nc.sbuf_tensor("B0", [128, HALF], DT))
        B1 = ctx.enter_context(nc.sbuf_tensor("B1", [128, HALF], DT))
        C0 = ctx.enter_context(nc.sbuf_tensor("C0", [128, HALF], DT))
        C1 = ctx.enter_context(nc.sbuf_tensor("C1", [128, HALF], DT))
        ps = ctx.enter_context(nc.psum_tensor("ps", [128, HALF], DT))

        A = [A0, A1]
        B = [B0, B1]
        C = [C0, C1]

        @block.sync
        def _(sync):
            for t, src in [(t_eta, eta), (t_etb, etb), (t_la, la),
                           (t_lb, lb), (t_ubc, ubc), (t_svec, svec)]:
                sync.dma_start(out=t[:], in_=src[:]).then_inc(dma_in, 16)
            for p in range(TOT):
                m, c0 = phase_mcol(p)
                s = p % 2
                sync.wait_ge(o_s, p + 1)
                sync.dma_start(
                    out=dists_o[m * 128:(m + 1) * 128, c0:c0 + HALF],
                    in_=C[s][:]).then_inc(dma_o, 16)
                sync.dma_start(
                    out=probs_o[m * 128:(m + 1) * 128, c0:c0 + HALF],
                    in_=A[s][:]).then_inc(dma_o, 16)
            sync.wait_ge(dma_o, 32 * TOT)

        @block.tensor
        def _(te):
            te.wait_ge(dma_in, NIN)
            for p in range(TOT):
                m, c0 = phase_mcol(p)
                if p >= 1:
                    te.wait_ge(dr_s, p)
                wla = t_la[:, m * 128:(m + 1) * 128]
                wlb = t_lb[:, m * 128:(m + 1) * 128]
                for sub in range(HALF // 512):
                    n0 = c0 + sub * 512
                    psl = ps[:, sub * 512:(sub + 1) * 512]
                    te.matmul(psl, wla, t_eta[:, n0:n0 + 512],
                              start=True, stop=False)
                    mm = te.matmul(psl, wlb, t_etb[:, n0:n0 + 512],
                                   start=False, stop=True)
                mm.then_inc(pe_s, 1)

        @block.vector
        def _(v):
            v.wait_ge(dma_in, NIN)
            for p in range(TOT + 1):
                s = p % 2
                if p < TOT:
                    m, c0 = phase_mcol(p)
                    if p >= 2:
                        v.wait_ge(dma_o, 32 * (p - 1))
                    v.wait_ge(pe_s, p + 1)
                    v.scalar_tensor_tensor(
                        out=A[s][:], in0=t_ubc[:, c0:c0 + HALF], scalar=0.0,
                        in1=ps[:], op0=OP.add, op1=OP.add)
                    v.scalar_tensor_tensor(
                        out=B[s][:], in0=t_ubc[:, c0:c0 + HALF],
                        scalar=t_svec[:, m:m + 1],
                        in1=ps[:], op0=OP.mult, op1=OP.add).then_inc(dr_s, 1)
                if p >= 1:
                    q = p - 1
                    sq = q % 2
                    v.wait_ge(ln_s, q + 1)
                    v.tensor_sub(out=A[sq][:], in0=A[sq][:],
                                 in1=B[sq][:]).then_inc(h_s, 1)

        @block.gpsimd
        def _(gp):
            for q in range(TOT):
                sq = q % 2
                gp.wait_ge(z_s, q + 1)
                gp.tensor_sub(out=C[sq][:], in0=C[sq][:], in1=A[sq][:])
                gp.tensor_scalar(
                    out=A[sq][:], in0=B[sq][:], scalar1=-0.5, scalar2=0.5,
                    op0=OP.mult, op1=OP.add).then_inc(o_s, 1)
            if bench:
                gp.wait_ge(o_s, TOT)
                gp.memset(A[0][:, 0:4], 0.0)
                gp.dma_start(out=done_o[:],
                             in_=A[0][:, 0:4]).then_inc(dma_o, 16)

        @block.scalar
        def _(sc):
            sc.wait_ge(dma_in, NIN)
            for p in range(TOT + 1):
                s = p % 2
                if p < TOT:
                    m, c0 = phase_mcol(p)
                    sc.wait_ge(dr_s, p + 1)
                    sc.activation(A[s][:], A[s][:], F.Ln,
                                  bias=t_svec[:, m:m + 1], scale=1.0)
                    sc.activation(B[s][:], B[s][:], F.Ln,
                                  bias=1.0, scale=1.0).then_inc(ln_s, 1)
                if p >= 1:
                    q = p - 1
                    sq = q % 2
                    sc.wait_ge(h_s, q + 1)
                    if q >= 2:
                        sc.wait_ge(dma_o, 32 * (q - 1))
                    sc.activation(B[sq][:], A[sq][:], F.Exp,
                                  bias=0.0, scale=0.5)
                    sc.activation(C[sq][:], B[sq][:], F.Ln,
                                  bias=1.0, scale=1.0)
                    sc.activation(A[sq][:], B[sq][:], F.Ln,
                                  bias=1.0, scale=-1.0).then_inc(z_s, 1)


    return nc


def _prepare_in_maps_raw(embeddings):
    E = np.ascontiguousarray(embeddings, dtype=np.float32)
    x2 = (E.astype(np.float64) ** 2).sum(axis=1)
    ET = E.T
    ETn2 = (-2.0 * ET).astype(np.float32)
    x2f = x2.astype(np.float32)

    eta = np.ascontiguousarray(ET[:128])
    etb = np.ascontiguousarray(ET[128:])
    ubc = np.ascontiguousarray(np.broadcast_to(x2f[None, :], (128, N_TOTAL)))

    in_maps = []
    for c in range(8):
        rs = slice(c * ROWS_PER_CORE, (c + 1) * ROWS_PER_CORE)
        sv = np.ascontiguousarray(x2f[rs].reshape(8, 128).T)  # svec[p, m]
        in_maps.append({
            "eta": eta, "etb": etb,
            "la": np.ascontiguousarray(ETn2[:128, rs]),
            "lb": np.ascontiguousarray(ETn2[128:, rs]),
            "ubc": ubc, "svec": sv,
        })
    return in_maps


def kernel(embeddings: np.ndarray) -> tuple[np.ndarray, np.ndarray]:
    global _compiled
    from concourse.bass_utils import run_bass_kernel_spmd

    if _compiled is None:
        _compiled = _build_raw()
    nc = _compiled

    in_maps = _prepare_in_maps_raw(embeddings)
    res = run_bass_kernel_spmd(nc, in_maps, list(range(8)))

    dists = np.empty((N_TOTAL, N_TOTAL), np.float32)
    probs = np.empty((N_TOTAL, N_TOTAL), np.float32)
    for c in range(8):
        rs = slice(c * ROWS_PER_CORE, (c + 1) * ROWS_PER_CORE)
        dists[rs] = res.results[c]["dists_o"]
        probs[rs] = res.results[c]["probs_o"]

    idx = np.arange(N_TOTAL)
    dists[idx, idx] = 0.0
    probs[idx, idx] = 0.0
    return (probs, dists)



# revision 3
# speedup vs baseline: 2.1256x; 2.1256x over previous
"""Poincare pairwise edge generator on 8 Trainium2 NeuronCores (v2).

Math: with c=1, mobius num2 factors as sqdist*D (D = 1 - 2<x,y> + s*u,
sqdist = s + u - 2<x,y>, s=|x|^2, u=|y|^2).  Dividing both by (1-u):
  S'' = sqdist/(1-u),  D'' = D/(1-u) = S'' + (1-s)
  z   = sqrt(S''/D'')
  dists = ln((1+z)/(1-z)) = 2*ln[(sqrt(S'') + sqrt(S''+(1-s))) / sqrt(1-s)]
  probs = sigmoid(-dists)
so the whole elementwise chain from the matmul output S'' is:
  v=Sqrt, vp=Sqrt(+bias), w=v+vp [DVE], L=Ln(w*rsq) [per-partition scale],
  probs=Sigmoid(-2L);  dists = 2L (host applies the *2 at unshard).

The matmul computes ps = 2^10 * (-2<x_i,x_j>)/(1-u_j) via ONE fp8e4m3
DoubleRow matmul per 512 columns (K=256 in one instruction, inputs
prescaled by 32 on each side).  The rank-1 terms (s_i + u_j)/(1-u_j) are
added during psum extraction: one DVE scalar_tensor_tensor per psum tile
(s_i term) + one tensor_add per tier (u term), using bf16 broadcast
tiles.  This environment charges ~25-130us PER INSTRUCTION regardless of
operand width (engines serialize), so the design minimizes instruction
count above all else.

Symmetry: only the upper triangle is computed (host mirrors).  Core c
processes global row-blocks {8t+c : t=0..7}; block 8t+c computes columns
[1024t, 8192) -- every core runs the IDENTICAL program (widths 8192,
7168, ..., 1024), only lhsT/scalar data differ.  Diagonal elements hit
sqrt of a tiny negative (fp8 noise) and come back NaN; the host triu
mirror discards them and zeroes the diagonal exactly.
"""

import sys

sys.path.insert(0, '/opt/trn_rl_repo')

import numpy as np

_compiled = None

N_TOTAL = 8192
N_BLOCKS = 64          # 128-row blocks globally
TIERS = 8              # per-core row-blocks; tier t width = 8192 - 1024t
SC = 32.0
SC2 = SC * SC          # 1024 = 2^10

TIER_W = [N_TOTAL - 1024 * t for t in range(TIERS)]
TIER_C0 = [1024 * t for t in range(TIERS)]


def _tiles():
    """[(tier, xoff, w)] psum tiles, w <= 4096."""
    out = []
    for t in range(TIERS):
        W = TIER_W[t]
        if W > 4096:
            out.append((t, 0, 4096))
            out.append((t, 4096, W - 4096))
        else:
            out.append((t, 0, W))
    return out


def _build_raw(reps=1, bench=False, tiny_io=False):
    import concourse.bass as bass
    import concourse.mybir as mybir

    DT = mybir.dt.float32
    BF = mybir.dt.bfloat16
    F8 = mybir.dt.float8e4
    F = mybir.ActivationFunctionType
    OP = mybir.AluOpType
    DR = mybir.MatmulPerfMode.DoubleRow

    nc = bass.Bass()

    if tiny_io:
        nc.declare_dram_parameter("tiny", [128, 4], DT, isOutput=False)
        lhsT = nc.dram_tensor("lhsT", [128, 2, 1024], F8)
        rhs = nc.dram_tensor("rhs", [128, 2, N_TOTAL], F8)
        bc1 = nc.dram_tensor("bc1", [128, N_TOTAL], BF)
        bc3 = nc.dram_tensor("bc3", [128, N_TOTAL], BF)
        sv = nc.dram_tensor("sv", [128, TIERS, 4], DT)
        d_o = [nc.dram_tensor(f"d{t}", [128, TIER_W[t]], BF)
               for t in range(TIERS)]
        p_o = [nc.dram_tensor(f"p{t}", [128, TIER_W[t]], BF)
               for t in range(TIERS)]
        done_o = nc.declare_dram_parameter("done_o", [128, 4], DT,
                                           isOutput=True)
    else:
        lhsT = nc.declare_dram_parameter("lhsT", [128, 2, 1024], F8,
                                         isOutput=False)
        rhs = nc.declare_dram_parameter("rhs", [128, 2, N_TOTAL], F8,
                                        isOutput=False)
        bc1 = nc.declare_dram_parameter("bc1", [128, N_TOTAL], BF,
                                        isOutput=False)
        bc3 = nc.declare_dram_parameter("bc3", [128, N_TOTAL], BF,
                                        isOutput=False)
        sv = nc.declare_dram_parameter("sv", [128, TIERS, 4], DT,
                                       isOutput=False)
        d_o = [nc.declare_dram_parameter(f"d{t}", [128, TIER_W[t]], BF,
                                         isOutput=True)
               for t in range(TIERS)]
        p_o = [nc.declare_dram_parameter(f"p{t}", [128, TIER_W[t]], BF,
                                         isOutput=True)
               for t in range(TIERS)]
        done_o = None

    NIN = 5 * 16
    tiles = _tiles()
    NTILE = len(tiles)             # 12
    # first global-tile index of each tier
    tier_first = {}
    for k, (t, xoff, w) in enumerate(tiles):
        tier_first.setdefault(t, k)

    from contextlib import ExitStack
    with ExitStack() as ctx:
        block = ctx.enter_context(nc.Block())
        dma_in = ctx.enter_context(nc.semaphore("dma_in"))
        pe_s = ctx.enter_context(nc.semaphore("pe_s"))
        x_s = ctx.enter_context(nc.semaphore("x_s"))
        xb_s = ctx.enter_context(nc.semaphore("xb_s"))
        v_s = ctx.enter_context(nc.semaphore("v_s"))
        w_s = ctx.enter_context(nc.semaphore("w_s"))
        o_s = ctx.enter_context(nc.semaphore("o_s"))
        dma_o = ctx.enter_context(nc.semaphore("dma_o"))
        t_l = ctx.enter_context(nc.sbuf_tensor("t_l", [128, 2, 1024], F8))
        t_r = ctx.enter_context(nc.sbuf_tensor("t_r", [128, 2, N_TOTAL], F8))
        t_b1 = ctx.enter_context(nc.sbuf_tensor("t_b1", [128, N_TOTAL], BF))
        t_b3 = ctx.enter_context(nc.sbuf_tensor("t_b3", [128, N_TOTAL], BF))
        t_sv = ctx.enter_context(nc.sbuf_tensor("t_sv", [128, TIERS, 4], DT))
        X = ctx.enter_context(nc.sbuf_tensor("X", [128, N_TOTAL], DT))
        V = ctx.enter_context(nc.sbuf_tensor("V", [128, N_TOTAL], DT))
        VP = ctx.enter_context(nc.sbuf_tensor("VP", [128, N_TOTAL], DT))
        LB = ctx.enter_context(nc.sbuf_tensor("LB", [128, N_TOTAL], BF))
        PB = ctx.enter_context(nc.sbuf_tensor("PB", [128, N_TOTAL], BF))
        ps = ctx.enter_context(nc.psum_tensor("ps", [128, 4096], DT))

        @block.sync
        def _(sync):
            for t, src in [(t_l, lhsT), (t_r, rhs), (t_b1, bc1),
                           (t_b3, bc3), (t_sv, sv)]:
                sync.dma_start(out=t[:], in_=src[:]).then_inc(dma_in, 16)
            for r in range(reps):
                for t in range(TIERS):
                    h = r * TIERS + t
                    W = TIER_W[t]
                    sync.wait_ge(o_s, h + 1)
                    sync.dma_start(out=d_o[t][:],
                                   in_=LB[:, :W]).then_inc(dma_o, 16)
                    sync.dma_start(out=p_o[t][:],
                                   in_=PB[:, :W]).then_inc(dma_o, 16)
            sync.wait_ge(dma_o, 32 * TIERS * reps + (16 if bench else 0))

        @block.tensor
        def _(te):
            te.wait_ge(dma_in, NIN)
            for r in range(reps):
                for k, (t, xoff, w) in enumerate(tiles):
                    g = r * NTILE + k
                    if g >= 1:
                        te.wait_ge(x_s, g)
                    lsl = t_l[:, :, t * 128:(t + 1) * 128]
                    for s in range(w // 512):
                        col = TIER_C0[t] + xoff + s * 512
                        mm = te.matmul(ps[:, s * 512:(s + 1) * 512],
                                       lsl, t_r[:, :, col:col + 512],
                                       start=True, stop=True, perf_mode=DR)
                    mm.then_inc(pe_s, 1)

        @block.vector
        def _(v):
            for r in range(reps):
                g = r * NTILE
                for t in range(TIERS):
                    h = r * TIERS + t
                    W = TIER_W[t]
                    c0 = TIER_C0[t]
                    if h >= 1:
                        v.wait_ge(o_s, h)     # X free (Ln of prev tier done)
                    tier_tiles = [tl for tl in tiles if tl[0] == t]
                    for (tt, xoff, w) in tier_tiles:
                        v.wait_ge(pe_s, g + 1)
                        v.scalar_tensor_tensor(
                            out=X[:, xoff:xoff + w],
                            in0=t_b1[:, c0 + xoff:c0 + xoff + w],
                            scalar=t_sv[:, t, 0:1],
                            in1=ps[:, 0:w],
                            op0=OP.mult, op1=OP.add).then_inc(x_s, 1)
                        g += 1
                    v.tensor_add(out=X[:, :W], in0=X[:, :W],
                                 in1=t_b3[:, c0:c0 + W]).then_inc(xb_s, 1)
                    v.wait_ge(v_s, 2 * (h + 1))
                    v.tensor_add(out=X[:, :W], in0=V[:, :W],
                                 in1=VP[:, :W]).then_inc(w_s, 1)

        @block.scalar
        def _(sc):
            for r in range(reps):
                for t in range(TIERS):
                    h = r * TIERS + t
                    W = TIER_W[t]
                    sc.wait_ge(xb_s, h + 1)
                    sc.activation(V[:, :W], X[:, :W], F.Sqrt,
                                  bias=0.0, scale=1.0 / SC2)
                    sc.activation(VP[:, :W], X[:, :W], F.Sqrt,
                                  bias=t_sv[:, t, 1:2],
                                  scale=1.0 / SC2).then_inc(v_s, 2)
                    sc.wait_ge(w_s, h + 1)
                    if h >= 1:
                        sc.wait_ge(dma_o, 32 * h)
                    sc.activation(LB[:, :W], X[:, :W], F.Ln,
                                  bias=0.0, scale=t_sv[:, t, 2:3])
                    sc.activation(PB[:, :W], LB[:, :W],
                                  F.Sigmoid, bias=0.0,
                                  scale=-2.0).then_inc(o_s, 1)

        @block.gpsimd
        def _(gp):
            if bench:
                gp.wait_ge(o_s, TIERS * reps)
                gp.memset(V[:, 0:4], 0.0)
                gp.dma_start(out=done_o[:],
                             in_=V[:, 0:4]).then_inc(dma_o, 16)

    return nc


def _prepare_in_maps(embeddings):
    import ml_dtypes
    f8 = ml_dtypes.float8_e4m3
    bf = ml_dtypes.bfloat16

    E = np.ascontiguousarray(embeddings, dtype=np.float32)
    u = (E.astype(np.float64) ** 2).sum(axis=1)          # [n]
    inv1u = 1.0 / (1.0 - u)

    ET = E.T.astype(np.float64)                          # [256, n]
    rhs = np.ascontiguousarray(
        (SC * ET * inv1u[None, :]).astype(np.float32).astype(f8)
        .reshape(2, 128, N_TOTAL).transpose(1, 0, 2))
    lhs_all = (-2.0 * SC * ET).astype(np.float32).astype(f8)   # [256, n]

    bc1 = np.ascontiguousarray(np.broadcast_to(
        (SC2 * inv1u).astype(np.float32).astype(bf)[None, :],
        (128, N_TOTAL)))
    bc3 = np.ascontiguousarray(np.broadcast_to(
        (SC2 * u * inv1u).astype(np.float32).astype(bf)[None, :],
        (128, N_TOTAL)))

    in_maps = []
    for c in range(8):
        gidx = np.concatenate(
            [np.arange((8 * t + c) * 128, (8 * t + c) * 128 + 128)
             for t in range(TIERS)])
        lhsT = np.ascontiguousarray(
            lhs_all[:, gidx].reshape(2, 128, 1024).transpose(1, 0, 2))
        sv = np.zeros((128, TIERS, 4), np.float32)
        for t in range(TIERS):
            sblk = u[(8 * t + c) * 128:(8 * t + c) * 128 + 128]
            sv[:, t, 0] = sblk
            sv[:, t, 1] = 1.0 - sblk
            sv[:, t, 2] = 1.0 / np.sqrt(1.0 - sblk)
        in_maps.append({"lhsT": lhsT, "rhs": rhs, "bc1": bc1, "bc3": bc3,
                        "sv": np.ascontiguousarray(sv)})
    return in_maps


def kernel(embeddings: np.ndarray) -> tuple[np.ndarray, np.ndarray]:
    global _compiled
    from concourse.bass_utils import run_bass_kernel_spmd

    if _compiled is None:
        _compiled = _build_raw()
    nc = _compiled

    in_maps = _prepare_in_maps(embeddings)
    res = run_bass_kernel_spmd(nc, in_maps, list(range(8)))

    dists = np.zeros((N_TOTAL, N_TOTAL), np.float32)
    probs = np.zeros((N_TOTAL, N_TOTAL), np.float32)
    for c in range(8):
        for t in range(TIERS):
            r0 = (8 * t + c) * 128
            c0 = TIER_C0[t]
            dists[r0:r0 + 128, c0:] = \
                np.asarray(res.results[c][f"d{t}"]).astype(np.float32) * 2.0
            probs[r0:r0 + 128, c0:] = \
                np.asarray(res.results[c][f"p{t}"]).astype(np.float32)

    du = np.triu(dists, 1)
    dists = du + du.T
    pu = np.triu(probs, 1)
    probs = pu + pu.T
    return (probs, dists)
